# revision 6
# baseline (speedup 1.0000x reference)
"""Trainium2 Bass kernel for CoAttention_TextImage.

Math: in both co-attention stages the query-side score is constant along
the softmax axis, so it cancels inside softmax:
  att_img[b,s,:]  = softmax(tanh(img[b]@W_i1)@w_a1[H:])  @ img[b]
  att_text[b,s,:] = softmax(tanh(text[b]@W_t2)@w_a2[H:]) @ text[b]
Each output is one per-batch vector broadcast over S.

Sharding: 8 cores, one uniform SPMD program. Cores 0-3 text side
(2 batches each, W=W_t2), cores 4-7 img side (W=W_i1, rows zero-padded
49->128; padded rows are excluded via a zeroed "validity" ones-column,
not an exp mask: pad rows have X=0 so score=0, e=1, but contribute 0 to
both u (X rows are zero) and Z (validity col is zero)).

v2 (vs the fp32r baseline at 19750ns): the baseline was paced by 14
serial dma_starts (650ns SP.SEQ each) + 8.8us of fp32 DMA transfer and
a long serial score tail. Changes:
- bf16 datapath: W/X/XT/wa shipped as bf16 (half the HBM bytes, PE
  stays 1 cycle/col). u is accumulated in fp32 PSUM from bf16 operands.
- Host pre-packs SBUF-image layouts (XT transposed for the score
  matmul, XN natural for the u matmul, WP in [k, half, ktile, n]
  order), so each input is ONE contiguous-per-partition DMA: 7 DMAs
  total (XT, 4x W quarter, XN, wa row) instead of 14.
- No PE transposes: XT comes from the host.
- Score = tensor_tensor_reduce on DVE (fused mult+reduce), chained
  across column halves via the accum initial-value operand.
- W DMA'd in 4 quarters (half-major) so stage-1 matmuls start ~2us
  after the XT chunk lands and overlap the remaining transfers.
- wa row DMA + output DMA issue from the Pool/SWDGE path (25ns seq
  issue; Pool engine is otherwise idle), keeping SP.SEQ for the 6
  input DMAs.
Host divides u/Z and broadcasts over S during unshard.
"""

import sys

if "/opt/trn_rl_repo" not in sys.path:
    sys.path.insert(0, "/opt/trn_rl_repo")

import numpy as np
import ml_dtypes

import concourse.bass as bass
import concourse.bacc as bacc
import concourse.tile as tile
from concourse import mybir
from concourse.bass_utils import run_bass_kernel_spmd

F32 = mybir.dt.float32
BF16 = mybir.dt.bfloat16
NPBF16 = ml_dtypes.bfloat16
B, S, R, H = 8, 128, 49, 768
KT = H // 128   # 6 contraction tiles
SEGS = 2        # batches per core
NH = 2          # column halves of 384
NCORES = 8
ALU = mybir.AluOpType
AF = mybir.ActivationFunctionType

_cache = {}


def build_program():
    if "nc" in _cache:
        return _cache["nc"]

    nc = bacc.Bacc("TRN2", target_bir_lowering=False, debug=False)

    # Host-packed DRAM images (already in SBUF layout, contiguous per row):
    #   XT[k, s*768 + kt*128 + r] = X[s, r, kt*128 + k]   (score lhsT)
    #   XN[r, s*770 + h] = X[s, r, h]; cols 768,769 = row-validity  (u rhs)
    #   WP[k, nh*2304 + kt*384 + n] = W[kt*128 + k, nh*384 + n]
    XT = nc.dram_tensor("XT", [128, SEGS * H], BF16, kind="ExternalInput")
    XN = nc.dram_tensor("XN", [128, SEGS * 770], BF16, kind="ExternalInput")
    WP = nc.dram_tensor("WP", [128, NH * KT * 384], BF16, kind="ExternalInput")
    WAR = nc.dram_tensor("WAR", [1, H], BF16, kind="ExternalInput")
    V = nc.dram_tensor("V", [SEGS, 770], F32, kind="ExternalOutput")

    with tile.TileContext(nc) as tc:
        with (
            tc.tile_pool(name="data", bufs=1) as data,
            tc.tile_pool(name="ypsum", bufs=1, space="PSUM") as ypsum,
            tc.tile_pool(name="upsum", bufs=2, space="PSUM") as upsum,
        ):
            xt = data.tile([128, SEGS, KT, 128], BF16)
            xn = data.tile([128, SEGS, 770], BF16)
            wp = data.tile([128, NH, KT, 384], BF16)
            war = data.tile([1, H], BF16)
            wab = data.tile([128, H], BF16)
            t1 = data.tile([128, SEGS, H], BF16)
            prodf = data.tile([128, SEGS, H], BF16)  # weighted products
            ssc = data.tile([128, SEGS, 4], F32)    # quarter score partials
            esc = data.tile([128, SEGS], BF16)
            usb = data.tile([1, SEGS, 770], F32)

            # wa row issued from the ACT sequencer: its 650ns issue slot
            # would otherwise delay XT/W on SP by one slot
            nc.scalar.dma_start(out=war[:], in_=WAR[:])

            # input DMAs from the SP sequencer, critical-path order:
            # XT, then W quarter-chunks half-major, then XN (needed late)
            nc.sync.dma_start(out=xt[:], in_=XT[:])
            for nh in range(NH):
                for g in range(2):
                    c0 = nh * (KT * 384) + g * (3 * 384)
                    nc.sync.dma_start(
                        out=wp[:, nh, 3 * g : 3 * g + 3, :],
                        in_=WP[:, c0 : c0 + 3 * 384],
                    )
            nc.sync.dma_start(out=xn[:], in_=XN[:])

            # wa broadcast to 128 partitions: ones-column matmul (exact)
            ones_f = data.tile([1, 128], F32)
            nc.vector.memset(ones_f[:], 1.0)
            ones_col = data.tile([1, 128], BF16)
            nc.vector.tensor_copy(out=ones_col[:], in_=ones_f[:])
            for nh in range(NH):
                # share y[0][nh]'s PSUM bank: consumed (copied to wab)
                # before stage-1 writes y00/y01, Tile adds the WAR dep
                wps = ypsum.tile([128, 384], F32, name=f"wps{nh}", tag=f"y0{nh}")
                nc.tensor.matmul(
                    wps[:], lhsT=ones_col[:],
                    rhs=war[:, nh * 384 : (nh + 1) * 384],
                    start=True, stop=True,
                )
                nc.vector.tensor_copy(
                    out=wab[:, nh * 384 : (nh + 1) * 384], in_=wps[:]
                )

            # stage 1, half-major: Y[s][nh] = X_seg @ W[:, half] (bf16, fp32 acc)
            y = [
                [
                    ypsum.tile([128, 384], F32, name=f"y{s}{nh}", tag=f"y{s}{nh}")
                    for nh in range(NH)
                ]
                for s in range(SEGS)
            ]
            for nh in range(NH):
                for g in range(2):
                    for s in range(SEGS):
                        for kt in range(3 * g, 3 * g + 3):
                            nc.tensor.matmul(
                                y[s][nh][:],
                                lhsT=xt[:, s, kt, :],
                                rhs=wp[:, nh, kt, :],
                                start=(kt == 0),
                                stop=(kt == KT - 1),
                            )
                for s in range(SEGS):
                    for q in range(2):
                        c0 = nh * 384 + q * 192
                        nc.scalar.activation(
                            out=t1[:, s, c0 : c0 + 192],
                            in_=y[s][nh][:, q * 192 : (q + 1) * 192],
                            func=AF.Tanh,
                        )
                        nc.vector.tensor_tensor(
                            out=prodf[:, s, c0 : c0 + 192],
                            in0=t1[:, s, c0 : c0 + 192],
                            in1=wab[:, c0 : c0 + 192],
                            op=ALU.mult,
                        )
                        nc.vector.tensor_reduce(
                            out=ssc[:, s, 2 * nh + q : 2 * nh + q + 1],
                            in_=prodf[:, s, c0 : c0 + 192],
                            axis=mybir.AxisListType.X, op=ALU.add,
                        )

            sfin = data.tile([128, SEGS], F32)
            for s in range(SEGS):
                nc.vector.tensor_reduce(
                    out=sfin[:, s : s + 1], in_=ssc[:, s, :],
                    axis=mybir.AxisListType.X, op=ALU.add,
                )
                nc.scalar.activation(
                    out=esc[:, s : s + 1],
                    in_=sfin[:, s : s + 1],
                    func=AF.Exp,
                )
            for s in range(SEGS):
                # u = e.T @ [X | valid] -> u[0:768] unnormalized, u[768]=Z
                u0 = upsum.tile([1, 512], F32, name=f"u0{s}", tag="u0")
                u1 = upsum.tile([1, 258], F32, name=f"u1{s}", tag="u1")
                nc.tensor.matmul(
                    u0[:], lhsT=esc[:, s : s + 1], rhs=xn[:, s, 0:512],
                    start=True, stop=True,
                )
                nc.tensor.matmul(
                    u1[:], lhsT=esc[:, s : s + 1], rhs=xn[:, s, 512:770],
                    start=True, stop=True,
                )
                nc.scalar.copy(out=usb[:, s, 0:512], in_=u0[:])
                nc.vector.tensor_copy(out=usb[:, s, 512:770], in_=u1[:])
            nc.gpsimd.dma_start(out=V[:], in_=usb[0:1, :, 0:770])

    nc.compile()
    _cache["nc"] = nc
    return nc


def _pack_core(X, valid, Wside, wa):
    """Build one core's host-packed inputs. X: (SEGS,128,H) f32,
    valid: (SEGS,128) f32, Wside: (H,H) f32, wa: (H,) f32."""
    xt = np.empty((128, SEGS * H), np.float32)
    xn = np.zeros((128, SEGS * 770), np.float32)
    for s in range(SEGS):
        A = X[s]                                   # (128, 768)
        xt[:, s * H : (s + 1) * H] = (
            A.reshape(128, KT, 128).transpose(2, 1, 0).reshape(128, H)
        )
        xn[:, s * 770 : s * 770 + H] = A
        xn[:, s * 770 + H : s * 770 + 770] = valid[s][:, None]
    wpk = (
        Wside.reshape(KT, 128, NH, 384)
        .transpose(1, 2, 0, 3)
        .reshape(128, NH * KT * 384)
    )
    return {
        "XT": xt.astype(NPBF16),
        "XN": xn.astype(NPBF16),
        "WP": np.ascontiguousarray(wpk).astype(NPBF16),
        "WAR": wa[None, :].astype(NPBF16),
    }


def make_in_maps(text, img, W_t2, W_i1, wa2, wa1):
    """Per-core input dicts. Cores 0-3: text side; cores 4-7: img side."""
    in_maps = []
    valid_t = np.ones((SEGS, 128), np.float32)
    valid_i = np.zeros((SEGS, 128), np.float32)
    valid_i[:, :R] = 1.0
    for c in range(4):
        in_maps.append(_pack_core(text[2 * c : 2 * c + 2], valid_t, W_t2, wa2))
    for c in range(4):
        Xp = np.zeros((SEGS, 128, H), np.float32)
        Xp[:, :R, :] = img[2 * c : 2 * c + 2]
        in_maps.append(_pack_core(Xp, valid_i, W_i1, wa1))
    return in_maps


def kernel(**inputs):
    text = np.ascontiguousarray(np.asarray(inputs["text_features"], np.float32))
    img = np.ascontiguousarray(np.asarray(inputs["img_features"], np.float32))
    W_t2 = np.ascontiguousarray(np.asarray(inputs["W_t2"], np.float32))
    W_i1 = np.ascontiguousarray(np.asarray(inputs["W_i1"], np.float32))
    wa2 = np.ascontiguousarray(np.asarray(inputs["w_a2"], np.float32)[H:])
    wa1 = np.ascontiguousarray(np.asarray(inputs["w_a1"], np.float32)[H:])

    nc = build_program()
    in_maps = make_in_maps(text, img, W_t2, W_i1, wa2, wa1)
    res = run_bass_kernel_spmd(nc, in_maps, core_ids=list(range(NCORES)))

    u = np.stack([np.asarray(r["V"], np.float32) for r in res.results])  # (8,2,770)
    v = u[:, :, 0:H] / u[:, :, H : H + 1]
    v_text = v[:4].reshape(B, H)
    v_img = v[4:].reshape(B, H)
    att_text = np.broadcast_to(v_text[:, None, :], (B, S, H)).copy()
    att_img = np.broadcast_to(v_img[:, None, :], (B, S, H)).copy()
    return att_text, att_img


# revision 8
# speedup vs baseline: 1.0817x; 1.0817x over previous
"""Trainium2 Bass kernel for CoAttention_TextImage.

Math: in both co-attention stages the query-side score is constant along
the softmax axis, so it cancels inside softmax:
  att_img[b,s,:]  = softmax(tanh(img[b]@W_i1)@w_a1[H:])  @ img[b]
  att_text[b,s,:] = softmax(tanh(text[b]@W_t2)@w_a2[H:]) @ text[b]
Each output is one per-batch vector broadcast over S.

Sharding: 8 cores, one uniform SPMD program. Cores 0-3 text side
(2 batches each, W=W_t2), cores 4-7 img side (W=W_i1, rows zero-padded
49->128; padded rows are excluded via a zeroed "validity" ones-column:
pad rows have X=0 so score=0, e=1, but contribute 0 to both u (zero X
rows) and Z (zero validity)).

v4 (19750 -> ~16000 -> this): the score matmul X@W dominates neither
accuracy nor output precision (it sits inside a softmax over ~uniform
scores), so it runs in fp8-e4m3 with DoubleRow packing (2 k-slices per
pass, 0.5 cyc/col): stage-1 PE time 0.96us, W DMA 0.6MB/core. W is
pre-scaled by 8 on the host (fp8e4 normals start at 2^-6; raw weights
~N(0,0.02) would land subnormal) and compensated exactly via the tanh
activation's scale=1/8. The u = e.T @ [X|valid] matmul stays bf16
(output precision) with fp32 PSUM accumulation.

Other structure vs the fp32r baseline:
- Host pre-packs SBUF-image layouts (XT DoubleRow-transposed, XN
  natural bf16, WP in [k, half, kpair, i, n] order): each input is one
  contiguous-per-partition DMA; wa row issues from the ACT sequencer so
  SP's first 650ns issue slot goes to XT.
- Score tail: tanh per (seg, 384-half) on ACT; weighted products on
  DVE (bf16, 2 elem/cyc); the 384-wide reduces for the last-finishing
  seg run on DVE while seg 0's run on Pool, so the two segs' score
  chains don't serialize on one engine.
- Output: PSUM -> SBUF copies (ACT+DVE in parallel), one DMA for both
  segs issued via the Pool/SWDGE path (no 650+625 SP/HWDGE serial
  issue at the tail).
Host divides u/Z and broadcasts over S during unshard.
"""

import sys

if "/opt/trn_rl_repo" not in sys.path:
    sys.path.insert(0, "/opt/trn_rl_repo")

import numpy as np
import ml_dtypes

import concourse.bass as bass
import concourse.bacc as bacc
import concourse.tile as tile
from concourse import mybir
from concourse.bass_utils import run_bass_kernel_spmd

F32 = mybir.dt.float32
BF16 = mybir.dt.bfloat16
FP8 = mybir.dt.float8e4
NPBF16 = ml_dtypes.bfloat16
NPFP8 = ml_dtypes.float8_e4m3
B, S, R, H = 8, 128, 49, 768
KT = H // 128   # 6 contraction tiles
KP = KT // 2    # 3 DoubleRow k-pair tiles
SEGS = 2        # batches per core
NH = 2          # column halves of 384
NCORES = 8
ALU = mybir.AluOpType
AF = mybir.ActivationFunctionType

# fallback flags (bisection): DR=False -> plain fp8 matmuls;
# SCORE_FP8=False -> bf16 score path, no W prescale
DR = True
SCORE_FP8 = True
SC_DT = FP8 if SCORE_FP8 else BF16
NP_SC = NPFP8 if SCORE_FP8 else NPBF16
WSCALE = 8.0 if SCORE_FP8 else 1.0

_cache = {}


def build_program():
    if "nc" in _cache:
        return _cache["nc"]

    nc = bacc.Bacc("TRN2", target_bir_lowering=False, debug=False)

    # Host-packed DRAM images (contiguous per partition row):
    #   XT[k, s*768 + kp*256 + i*128 + r] = X[s, r, (2kp+i)*128 + k]
    #   XN[r, s*770 + h] = X[s, r, h]; cols 768,769 = row-validity
    #   WP[k, nh*2304 + kp*768 + i*384 + n] = WSCALE*W[(2kp+i)*128+k, nh*384+n]
    XT = nc.dram_tensor("XT", [128, SEGS * H], SC_DT, kind="ExternalInput")
    XN = nc.dram_tensor("XN", [128, SEGS * 770], BF16, kind="ExternalInput")
    WP = nc.dram_tensor("WP", [128, NH * KT * 384], SC_DT, kind="ExternalInput")
    WAR = nc.dram_tensor("WAR", [1, H], BF16, kind="ExternalInput")
    V = nc.dram_tensor("V", [SEGS, 770], F32, kind="ExternalOutput")

    with tile.TileContext(nc) as tc:
        with (
            tc.tile_pool(name="data", bufs=1) as data,
            tc.tile_pool(name="ypsum", bufs=1, space="PSUM") as ypsum,
            tc.tile_pool(name="upsum", bufs=2, space="PSUM") as upsum,
        ):
            xt = data.tile([128, SEGS, KP, 2, 128], SC_DT)
            xn = data.tile([128, SEGS, 770], BF16)
            wp = data.tile([128, NH, KP, 2, 384], SC_DT)
            war = data.tile([1, H], BF16)
            wab = data.tile([128, H], BF16)
            t1 = data.tile([128, SEGS, H], BF16)
            prodf = data.tile([128, SEGS, H], BF16)
            ssc = data.tile([128, SEGS, NH], F32)
            sfin = data.tile([128, SEGS], F32)
            esc = data.tile([128, SEGS], BF16)
            usb = data.tile([1, SEGS, 770], F32)

            # wa row from the ACT sequencer (SP's first slot goes to XT)
            nc.scalar.dma_start(out=war[:], in_=WAR[:])

            # SP input DMAs in critical-path order: XT, W halves, XN
            nc.sync.dma_start(out=xt[:], in_=XT[:])
            for nh in range(NH):
                nc.sync.dma_start(
                    out=wp[:, nh, :, :, :],
                    in_=WP[:, nh * (KT * 384) : (nh + 1) * (KT * 384)],
                )
            nc.sync.dma_start(out=xn[:], in_=XN[:])

            # wa broadcast to 128 partitions: ones-column matmul (exact)
            ones_f = data.tile([1, 128], F32)
            nc.vector.memset(ones_f[:], 1.0)
            ones_col = data.tile([1, 128], BF16)
            nc.vector.tensor_copy(out=ones_col[:], in_=ones_f[:])
            for nh in range(NH):
                # share y[0][nh]'s PSUM bank (consumed before stage-1)
                wps = ypsum.tile([128, 384], F32, name=f"wps{nh}", tag=f"y0{nh}")
                nc.tensor.matmul(
                    wps[:], lhsT=ones_col[:],
                    rhs=war[:, nh * 384 : (nh + 1) * 384],
                    start=True, stop=True,
                )
                nc.vector.tensor_copy(
                    out=wab[:, nh * 384 : (nh + 1) * 384], in_=wps[:]
                )

            # stage 1, half-major: Y[s][nh] = (X @ WSCALE*W)[:, half]
            y = [
                [
                    ypsum.tile([128, 384], F32, name=f"y{s}{nh}", tag=f"y{s}{nh}")
                    for nh in range(NH)
                ]
                for s in range(SEGS)
            ]
            for nh in range(NH):
                for s in range(SEGS):
                    if DR:
                        for kp in range(KP):
                            nc.tensor.matmul(
                                y[s][nh][:],
                                lhsT=xt[:, s, kp, :, :],
                                rhs=wp[:, nh, kp, :, :],
                                start=(kp == 0),
                                stop=(kp == KP - 1),
                                perf_mode=mybir.MatmulPerfMode.DoubleRow,
                            )
                    else:
                        for kp in range(KP):
                            for i in range(2):
                                nc.tensor.matmul(
                                    y[s][nh][:],
                                    lhsT=xt[:, s, kp, i, :],
                                    rhs=wp[:, nh, kp, i, :],
                                    start=(kp == 0 and i == 0),
                                    stop=(kp == KP - 1 and i == 1),
                                )
                for s in range(SEGS):
                    # tanh(Y/WSCALE) undoes the host-side W prescale exactly
                    nc.scalar.activation(
                        out=t1[:, s, nh * 384 : (nh + 1) * 384],
                        in_=y[s][nh][:],
                        func=AF.Tanh,
                        scale=1.0 / WSCALE,
                    )
                for s in range(SEGS):
                    # seg0 mult on Pool (hidden), keeping DVE for reduces
                    eng = nc.gpsimd if s == 0 else nc.vector
                    eng.tensor_tensor(
                        out=prodf[:, s, nh * 384 : (nh + 1) * 384],
                        in0=t1[:, s, nh * 384 : (nh + 1) * 384],
                        in1=wab[:, nh * 384 : (nh + 1) * 384],
                        op=ALU.mult,
                    )
            # per-half reduces: seg 1 (last to finish) on DVE, seg 0 on
            # the otherwise-idle Pool so the chains run in parallel
            for s in range(SEGS):
                for nh in range(NH):
                    nc.vector.tensor_reduce(
                        out=ssc[:, s, nh : nh + 1],
                        in_=prodf[:, s, nh * 384 : (nh + 1) * 384],
                        axis=mybir.AxisListType.X, op=ALU.add,
                    )
                nc.vector.tensor_tensor(
                    out=sfin[:, s : s + 1],
                    in0=ssc[:, s, 0:1], in1=ssc[:, s, 1:2], op=ALU.add,
                )
            for s in range(SEGS):
                nc.scalar.activation(
                    out=esc[:, s : s + 1],
                    in_=sfin[:, s : s + 1],
                    func=AF.Exp,
                )
            for s in range(SEGS):
                # u = e.T @ [X | valid] -> u[0:768] unnormalized, u[768]=Z
                u0 = upsum.tile([1, 512], F32, name=f"u0{s}", tag="u0")
                u1 = upsum.tile([1, 258], F32, name=f"u1{s}", tag="u1")
                nc.tensor.matmul(
                    u0[:], lhsT=esc[:, s : s + 1], rhs=xn[:, s, 0:512],
                    start=True, stop=True,
                )
                nc.tensor.matmul(
                    u1[:], lhsT=esc[:, s : s + 1], rhs=xn[:, s, 512:770],
                    start=True, stop=True,
                )
                nc.scalar.copy(out=usb[:, s, 0:512], in_=u0[:])
                nc.vector.tensor_copy(out=usb[:, s, 512:770], in_=u1[:])
            nc.gpsimd.dma_start(out=V[:], in_=usb[0:1, :, 0:770])

    nc.compile()
    _cache["nc"] = nc
    return nc


def _pack_core(X, valid, Wside, wa):
    """One core's host-packed inputs. X: (SEGS,128,H) f32,
    valid: (SEGS,128) f32, Wside: (H,H) f32, wa: (H,) f32."""
    xt = np.empty((128, SEGS * H), np.float32)
    xn = np.zeros((128, SEGS * 770), np.float32)
    for s in range(SEGS):
        A = X[s]                                   # (128, 768)
        # [k, kp, i, r] <- A[r, (2kp+i)*128+k]
        xt[:, s * H : (s + 1) * H] = (
            A.reshape(128, KP, 2, 128).transpose(3, 1, 2, 0).reshape(128, H)
        )
        xn[:, s * 770 : s * 770 + H] = A
        xn[:, s * 770 + H : s * 770 + 770] = valid[s][:, None]
    # [k, nh, kp, i, n] <- W[(2kp+i)*128+k, nh*384+n]
    wpk = (
        (WSCALE * Wside)
        .reshape(KP, 2, 128, NH, 384)
        .transpose(2, 3, 0, 1, 4)
        .reshape(128, NH * KT * 384)
    )
    return {
        "XT": xt.astype(NP_SC),
        "XN": xn.astype(NPBF16),
        "WP": np.ascontiguousarray(wpk).astype(NP_SC),
        "WAR": wa[None, :].astype(NPBF16),
    }


def make_in_maps(text, img, W_t2, W_i1, wa2, wa1):
    """Per-core input dicts. Cores 0-3: text side; cores 4-7: img side."""
    in_maps = []
    valid_t = np.ones((SEGS, 128), np.float32)
    valid_i = np.zeros((SEGS, 128), np.float32)
    valid_i[:, :R] = 1.0
    for c in range(4):
        in_maps.append(_pack_core(text[2 * c : 2 * c + 2], valid_t, W_t2, wa2))
    for c in range(4):
        Xp = np.zeros((SEGS, 128, H), np.float32)
        Xp[:, :R, :] = img[2 * c : 2 * c + 2]
        in_maps.append(_pack_core(Xp, valid_i, W_i1, wa1))
    return in_maps


def kernel(**inputs):
    text = np.ascontiguousarray(np.asarray(inputs["text_features"], np.float32))
    img = np.ascontiguousarray(np.asarray(inputs["img_features"], np.float32))
    W_t2 = np.ascontiguousarray(np.asarray(inputs["W_t2"], np.float32))
    W_i1 = np.ascontiguousarray(np.asarray(inputs["W_i1"], np.float32))
    wa2 = np.ascontiguousarray(np.asarray(inputs["w_a2"], np.float32)[H:])
    wa1 = np.ascontiguousarray(np.asarray(inputs["w_a1"], np.float32)[H:])

    nc = build_program()
    in_maps = make_in_maps(text, img, W_t2, W_i1, wa2, wa1)
    res = run_bass_kernel_spmd(nc, in_maps, core_ids=list(range(NCORES)))

    u = np.stack([np.asarray(r["V"], np.float32) for r in res.results])  # (8,2,770)
    v = u[:, :, 0:H] / u[:, :, H : H + 1]
    v_text = v[:4].reshape(B, H)
    v_img = v[4:].reshape(B, H)
    att_text = np.broadcast_to(v_text[:, None, :], (B, S, H)).copy()
    att_img = np.broadcast_to(v_img[:, None, :], (B, S, H)).copy()
    return att_text, att_img


# revision 9
# speedup vs baseline: 1.1801x; 1.0910x over previous
"""Trainium2 Bass kernel for CoAttention_TextImage.

Math: in both co-attention stages the query-side score is constant along
the softmax axis, so it cancels inside softmax:
  att_img[b,s,:]  = softmax(tanh(img[b]@W_i1)@w_a1[H:])  @ img[b]
  att_text[b,s,:] = softmax(tanh(text[b]@W_t2)@w_a2[H:]) @ text[b]
Each output is one per-batch vector broadcast over S.

Sharding: 8 cores, one uniform SPMD program. Cores 0-3 text side
(2 batches each, W=W_t2), cores 4-7 img side (W=W_i1, rows zero-padded
49->128; padded rows are excluded via a zeroed "validity" ones-column:
pad rows have X=0 so score=0, e=1, but contribute 0 to both u (zero X
rows) and Z (zero validity)).

v4 (19750 -> ~16000 -> this): the score matmul X@W dominates neither
accuracy nor output precision (it sits inside a softmax over ~uniform
scores), so it runs in fp8-e4m3 with DoubleRow packing (2 k-slices per
pass, 0.5 cyc/col): stage-1 PE time 0.96us, W DMA 0.6MB/core. W is
pre-scaled by 8 on the host (fp8e4 normals start at 2^-6; raw weights
~N(0,0.02) would land subnormal) and compensated exactly via the tanh
activation's scale=1/8. The u = e.T @ [X|valid] matmul stays bf16
(output precision) with fp32 PSUM accumulation.

Other structure vs the fp32r baseline:
- Host pre-packs SBUF-image layouts (XT DoubleRow-transposed, XN
  natural bf16, WP in [k, half, kpair, i, n] order): each input is one
  contiguous-per-partition DMA; wa row issues from the ACT sequencer so
  SP's first 650ns issue slot goes to XT.
- Score tail: tanh per (seg, 384-half) on ACT; weighted products on
  DVE (bf16, 2 elem/cyc); the 384-wide reduces for the last-finishing
  seg run on DVE while seg 0's run on Pool, so the two segs' score
  chains don't serialize on one engine.
- Output: PSUM -> SBUF copies (ACT+DVE in parallel), one DMA for both
  segs issued via the Pool/SWDGE path (no 650+625 SP/HWDGE serial
  issue at the tail).
Host divides u/Z and broadcasts over S during unshard.
"""

import sys

if "/opt/trn_rl_repo" not in sys.path:
    sys.path.insert(0, "/opt/trn_rl_repo")

import numpy as np
import ml_dtypes

import concourse.bass as bass
import concourse.bacc as bacc
import concourse.tile as tile
from concourse import mybir
from concourse.bass_utils import run_bass_kernel_spmd

F32 = mybir.dt.float32
BF16 = mybir.dt.bfloat16
FP8 = mybir.dt.float8e4
NPBF16 = ml_dtypes.bfloat16
NPFP8 = ml_dtypes.float8_e4m3
B, S, R, H = 8, 128, 49, 768
KT = H // 128   # 6 contraction tiles
KP = KT // 2    # 3 DoubleRow k-pair tiles
SEGS = 2        # batches per core
NH = 2          # column halves of 384
NCORES = 8
ALU = mybir.AluOpType
AF = mybir.ActivationFunctionType

# fallback flags (bisection): DR=False -> plain fp8 matmuls;
# SCORE_FP8=False -> bf16 score path, no W prescale
DR = True
SCORE_FP8 = True
SC_DT = FP8 if SCORE_FP8 else BF16
NP_SC = NPFP8 if SCORE_FP8 else NPBF16
WSCALE = 8.0 if SCORE_FP8 else 1.0

_cache = {}


def build_program():
    if "nc" in _cache:
        return _cache["nc"]

    nc = bacc.Bacc("TRN2", target_bir_lowering=False, debug=False)

    # Host-packed DRAM images (contiguous per partition row):
    #   XT[k, s*768 + kp*256 + i*128 + r] = X[s, r, (2kp+i)*128 + k]
    #   XN[r, s*770 + h] = X[s, r, h]; cols 768,769 = row-validity
    #   WP[k, nh*2304 + kp*768 + i*384 + n] = WSCALE*W[(2kp+i)*128+k, nh*384+n]
    XT = nc.dram_tensor("XT", [128, SEGS * H], SC_DT, kind="ExternalInput")
    XN = nc.dram_tensor("XN", [128, SEGS * 770], BF16, kind="ExternalInput")
    WP = nc.dram_tensor("WP", [128, NH * KT * 384], SC_DT, kind="ExternalInput")
    WAR = nc.dram_tensor("WAR", [1, H], BF16, kind="ExternalInput")
    V = nc.dram_tensor("V", [SEGS, 770], F32, kind="ExternalOutput")

    with tile.TileContext(nc) as tc:
        with (
            tc.tile_pool(name="data", bufs=1) as data,
            tc.tile_pool(name="ypsum", bufs=1, space="PSUM") as ypsum,
            tc.tile_pool(name="upsum", bufs=2, space="PSUM") as upsum,
        ):
            xt = data.tile([128, SEGS, KP, 2, 128], SC_DT)
            xn = data.tile([128, SEGS, 770], BF16)
            wp = data.tile([128, NH, KP, 2, 384], SC_DT)
            war = data.tile([1, H], BF16)
            wab = data.tile([128, H], BF16)
            t1 = data.tile([128, SEGS, H], BF16)
            prodf = [
                [data.tile([128, 384], BF16, name=f"pr{s}{nh}") for nh in range(NH)]
                for s in range(SEGS)
            ]
            ssc = [data.tile([128, NH], F32, name=f"ssc{s}") for s in range(SEGS)]
            sfin = [data.tile([128, 1], F32, name=f"sfin{s}") for s in range(SEGS)]
            esc = data.tile([128, SEGS], BF16)
            usb = data.tile([1, SEGS, 770], F32)

            # wa row via Pool/SWDGE: bypasses the HWDGE generator, which
            # otherwise serializes XT/W/XN gen by an extra 625ns slot
            nc.gpsimd.dma_start(out=war[:], in_=WAR[:])

            # SP input DMAs in critical-path order: XT, W halves, XN
            nc.sync.dma_start(out=xt[:], in_=XT[:])
            for nh in range(NH):
                nc.sync.dma_start(
                    out=wp[:, nh, :, :, :],
                    in_=WP[:, nh * (KT * 384) : (nh + 1) * (KT * 384)],
                )
            nc.sync.dma_start(out=xn[:], in_=XN[:])

            # wa broadcast to 128 partitions: ones-column matmul (exact)
            ones_f = data.tile([1, 128], F32)
            nc.vector.memset(ones_f[:], 1.0)
            ones_col = data.tile([1, 128], BF16)
            nc.vector.tensor_copy(out=ones_col[:], in_=ones_f[:])
            for nh in range(NH):
                # share y[0][nh]'s PSUM bank (consumed before stage-1)
                wps = ypsum.tile([128, 384], F32, name=f"wps{nh}", tag=f"y0{nh}")
                nc.tensor.matmul(
                    wps[:], lhsT=ones_col[:],
                    rhs=war[:, nh * 384 : (nh + 1) * 384],
                    start=True, stop=True,
                )
                nc.vector.tensor_copy(
                    out=wab[:, nh * 384 : (nh + 1) * 384], in_=wps[:]
                )

            # stage 1, half-major: Y[s][nh] = (X @ WSCALE*W)[:, half]
            y = [
                [
                    ypsum.tile([128, 384], F32, name=f"y{s}{nh}", tag=f"y{s}{nh}")
                    for nh in range(NH)
                ]
                for s in range(SEGS)
            ]
            for nh in range(NH):
                for s in range(SEGS):
                    if DR:
                        for kp in range(KP):
                            nc.tensor.matmul(
                                y[s][nh][:],
                                lhsT=xt[:, s, kp, :, :],
                                rhs=wp[:, nh, kp, :, :],
                                start=(kp == 0),
                                stop=(kp == KP - 1),
                                perf_mode=mybir.MatmulPerfMode.DoubleRow,
                            )
                    else:
                        for kp in range(KP):
                            for i in range(2):
                                nc.tensor.matmul(
                                    y[s][nh][:],
                                    lhsT=xt[:, s, kp, i, :],
                                    rhs=wp[:, nh, kp, i, :],
                                    start=(kp == 0 and i == 0),
                                    stop=(kp == KP - 1 and i == 1),
                                )
                for s in range(SEGS):
                    # tanh(Y/WSCALE) undoes the host-side W prescale exactly
                    nc.scalar.activation(
                        out=t1[:, s, nh * 384 : (nh + 1) * 384],
                        in_=y[s][nh][:],
                        func=AF.Tanh,
                        scale=1.0 / WSCALE,
                    )
                # seg0 mult on DVE (fast, its tanh lands first),
                # seg1 mult on Pool so the chains overlap
                nc.vector.tensor_tensor(
                    out=prodf[0][nh][:],
                    in0=t1[:, 0, nh * 384 : (nh + 1) * 384],
                    in1=wab[:, nh * 384 : (nh + 1) * 384],
                    op=ALU.mult,
                )
                nc.gpsimd.tensor_tensor(
                    out=prodf[1][nh][:],
                    in0=t1[:, 1, nh * 384 : (nh + 1) * 384],
                    in1=wab[:, nh * 384 : (nh + 1) * 384],
                    op=ALU.mult,
                )
                nc.vector.tensor_reduce(
                    out=ssc[0][:, nh : nh + 1], in_=prodf[0][nh][:],
                    axis=mybir.AxisListType.X, op=ALU.add,
                )
            nc.vector.tensor_tensor(
                out=sfin[0][:], in0=ssc[0][:, 0:1], in1=ssc[0][:, 1:2],
                op=ALU.add,
            )
            for nh in range(NH):
                nc.vector.tensor_reduce(
                    out=ssc[1][:, nh : nh + 1], in_=prodf[1][nh][:],
                    axis=mybir.AxisListType.X, op=ALU.add,
                )
            nc.vector.tensor_tensor(
                out=sfin[1][:], in0=ssc[1][:, 0:1], in1=ssc[1][:, 1:2],
                op=ALU.add,
            )
            # per-seg exp -> u -> copies, seg0 group first (ready first);
            # copies balanced ACT/DVE so neither engine runs two serially
            u0 = [None, None]
            u1 = [None, None]
            for s in range(SEGS):
                nc.scalar.activation(
                    out=esc[:, s : s + 1], in_=sfin[s][:], func=AF.Exp,
                )
                u0[s] = upsum.tile([1, 512], F32, name=f"u0{s}", tag="u0")
                u1[s] = upsum.tile([1, 258], F32, name=f"u1{s}", tag="u1")
                nc.tensor.matmul(
                    u0[s][:], lhsT=esc[:, s : s + 1], rhs=xn[:, s, 0:512],
                    start=True, stop=True,
                )
                nc.tensor.matmul(
                    u1[s][:], lhsT=esc[:, s : s + 1], rhs=xn[:, s, 512:770],
                    start=True, stop=True,
                )
                if s == 0:
                    nc.scalar.copy(out=usb[:, s, 0:512], in_=u0[s][:])
                    nc.vector.tensor_copy(out=usb[:, s, 512:770], in_=u1[s][:])
                else:
                    nc.vector.tensor_copy(out=usb[:, s, 0:512], in_=u0[s][:])
                    nc.scalar.copy(out=usb[:, s, 512:770], in_=u1[s][:])
            nc.gpsimd.dma_start(out=V[:], in_=usb[0:1, :, 0:770])

    nc.compile()
    _cache["nc"] = nc
    return nc


def _pack_core(X, valid, Wside, wa):
    """One core's host-packed inputs. X: (SEGS,128,H) f32,
    valid: (SEGS,128) f32, Wside: (H,H) f32, wa: (H,) f32."""
    xt = np.empty((128, SEGS * H), np.float32)
    xn = np.zeros((128, SEGS * 770), np.float32)
    for s in range(SEGS):
        A = X[s]                                   # (128, 768)
        # [k, kp, i, r] <- A[r, (2kp+i)*128+k]
        xt[:, s * H : (s + 1) * H] = (
            A.reshape(128, KP, 2, 128).transpose(3, 1, 2, 0).reshape(128, H)
        )
        xn[:, s * 770 : s * 770 + H] = A
        xn[:, s * 770 + H : s * 770 + 770] = valid[s][:, None]
    # [k, nh, kp, i, n] <- W[(2kp+i)*128+k, nh*384+n]
    wpk = (
        (WSCALE * Wside)
        .reshape(KP, 2, 128, NH, 384)
        .transpose(2, 3, 0, 1, 4)
        .reshape(128, NH * KT * 384)
    )
    return {
        "XT": xt.astype(NP_SC),
        "XN": xn.astype(NPBF16),
        "WP": np.ascontiguousarray(wpk).astype(NP_SC),
        "WAR": wa[None, :].astype(NPBF16),
    }


def make_in_maps(text, img, W_t2, W_i1, wa2, wa1):
    """Per-core input dicts. Cores 0-3: text side; cores 4-7: img side."""
    in_maps = []
    valid_t = np.ones((SEGS, 128), np.float32)
    valid_i = np.zeros((SEGS, 128), np.float32)
    valid_i[:, :R] = 1.0
    for c in range(4):
        in_maps.append(_pack_core(text[2 * c : 2 * c + 2], valid_t, W_t2, wa2))
    for c in range(4):
        Xp = np.zeros((SEGS, 128, H), np.float32)
        Xp[:, :R, :] = img[2 * c : 2 * c + 2]
        in_maps.append(_pack_core(Xp, valid_i, W_i1, wa1))
    return in_maps


def kernel(**inputs):
    text = np.ascontiguousarray(np.asarray(inputs["text_features"], np.float32))
    img = np.ascontiguousarray(np.asarray(inputs["img_features"], np.float32))
    W_t2 = np.ascontiguousarray(np.asarray(inputs["W_t2"], np.float32))
    W_i1 = np.ascontiguousarray(np.asarray(inputs["W_i1"], np.float32))
    wa2 = np.ascontiguousarray(np.asarray(inputs["w_a2"], np.float32)[H:])
    wa1 = np.ascontiguousarray(np.asarray(inputs["w_a1"], np.float32)[H:])

    nc = build_program()
    in_maps = make_in_maps(text, img, W_t2, W_i1, wa2, wa1)
    res = run_bass_kernel_spmd(nc, in_maps, core_ids=list(range(NCORES)))

    u = np.stack([np.asarray(r["V"], np.float32) for r in res.results])  # (8,2,770)
    v = u[:, :, 0:H] / u[:, :, H : H + 1]
    v_text = v[:4].reshape(B, H)
    v_img = v[4:].reshape(B, H)
    att_text = np.broadcast_to(v_text[:, None, :], (B, S, H)).copy()
    att_img = np.broadcast_to(v_img[:, None, :], (B, S, H)).copy()
    return att_text, att_img
